# revision 26
# baseline (speedup 1.0000x reference)
"""Trainium2 Bass kernel for nn_BiLinearDotLayer.

Computes, for feature (B,F,E)=(2048,200,64) f32 and weight (F,E,E):
    bilinear[b,i,d] = sum_e feature[b,i,e] * weight[i,e,d]
    out[b,i,j]      = sum_d bilinear[b,i,d] * feature[b,j,d]

Strategy (8 NeuronCores, data-parallel over batch):
  - Each core handles 256 batches; weight replicated.
  - Host pre-transposes feature to featT[e, b, i] and packs even/odd
    batches into SBUF partition halves (p*64+e) so all on-chip tiles use
    128 partitions (full DMA width) and the two batch parities run
    concurrently on the PE array's row-strips (K=64 each).
  - Both einsums run fully on-chip per block of 128 batches; only the
    feature shard and weight are read and only the final (256,200,200)
    f32 output is written per core.
  - Matmuls run in split-precision bf16 (x = hi + lo, three bf16
    matmuls hi*hi + hi*lo + lo*hi accumulated in fp32 PSUM, ~1e-5 rel
    error). bf16 streams at 1 cycle/row and, unlike fp32r, counts as PE
    activity for the HAM clock-gate, so the array stays at 2.4 GHz.
"""

import os
import sys

for _p in ("/opt/trn_rl_repo", "/root/.axon_site/_ro/trn_rl_repo"):
    if os.path.isdir(_p) and _p not in sys.path:
        sys.path.insert(0, _p)

import numpy as np

B, F, E = 2048, 200, 64
NCORES = 8
BLOC = B // NCORES            # 256 batches per core
NPAIR = BLOC // 2             # 128 even/odd batch pairs per core
BLOCKS = 2
PPB = NPAIR // BLOCKS         # 64 pairs (128 batches) per block
IGRP = 8                      # einsum1 i's per PSUM group (1 bank)
STG = 2                       # pairs per staged out-DMA (4 batches, 640KB)

_RUNNER = None


def _build_program():
    import concourse.tile as tile
    from concourse import bacc, mybir

    f32 = mybir.dt.float32
    bf16 = mybir.dt.bfloat16
    nc = bacc.Bacc("TRN2", target_bir_lowering=False, debug=False)

    # feature packed as interleaved bf16 hi/lo: fpk[p*64+e, bb*400+hl*200+i]
    fpk = nc.dram_tensor("fpk", [128, NPAIR * 2 * F], bf16, kind="ExternalInput")
    # weight packed as wpk[p*64+e, i*128 + hl*64 + d] (bf16 hi/lo interleaved)
    wpk = nc.dram_tensor("wpk", [128, F * 2 * E], bf16, kind="ExternalInput")
    # Device-friendly output layout: out_dev[p, b, ci, j] = out[b, 2p+ci, j].
    # Each partition's slice is contiguous in DRAM, so out-DMA descriptors are
    # one 12.8KB run per partition per stage group (vs 800B interleaved runs).
    # The host un-permutes afterwards.
    out = nc.dram_tensor("out", [100, BLOC, 2, F], f32, kind="ExternalOutput")
    out_v = out.ap()

    with tile.TileContext(nc) as tc:
        with (
            tc.tile_pool(name="wpool", bufs=3) as wpool,
            tc.tile_pool(name="fpool", bufs=2) as fpool,
            tc.tile_pool(name="bpool", bufs=1) as bpool,
            tc.tile_pool(name="stpool", bufs=4) as stpool,
            tc.tile_pool(name="ps1", bufs=2, space="PSUM") as ps1pool,
            tc.tile_pool(name="ps2", bufs=3, space="PSUM") as ps2pool,
        ):
            ftiles = [None] * BLOCKS
            btiles = [None] * BLOCKS  # each entry: (bhi, blo)

            def load_block(k):
                ftiles[k] = fpool.tile(
                    [128, PPB * 2 * F], bf16, name="ftile", tag="ftile"
                )
                nc.sync.dma_start(
                    out=ftiles[k][:],
                    in_=fpk.ap()[:, k * PPB * 2 * F : (k + 1) * PPB * 2 * F],
                )
                bhi = bpool.tile([128, PPB * F], bf16, name="bhi", tag="bhi")
                blo = bpool.tile([128, PPB * F], bf16, name="blo", tag="blo")
                btiles[k] = (bhi, blo)

            dma_i = 0  # alternates out-DMAs between sync and scalar rings

            def e1_group(k, i0):
                """einsum1 for i in [i0, i0+gs) of block k (bf16 hi/lo x3)."""
                gs = min(IGRP, F - i0)
                f4 = ftiles[k][:].rearrange(
                    "p (bb hl i) -> p bb hl i", hl=2, i=F
                )
                wseg = wpool.tile([128, IGRP * 2 * E], bf16, name="wseg", tag="wseg")
                nc.sync.dma_start(
                    out=wseg[:, : gs * 2 * E],
                    in_=wpk.ap()[:, i0 * 2 * E : (i0 + gs) * 2 * E],
                )
                pst = ps1pool.tile([128, IGRP * PPB], f32)
                for g in range(gs):
                    i = i0 + g
                    for p in (0, 1):
                        pr = slice(p * 64, (p + 1) * 64)
                        po = pst[pr, g * PPB : (g + 1) * PPB]
                        whi = wseg[pr, (g * 2) * E : (g * 2 + 1) * E]
                        wlo = wseg[pr, (g * 2 + 1) * E : (g * 2 + 2) * E]
                        fhi = f4[pr, :, 0, i]
                        flo = f4[pr, :, 1, i]
                        nc.tensor.matmul(out=po, lhsT=whi, rhs=fhi, start=True, stop=False)
                        nc.tensor.matmul(out=po, lhsT=whi, rhs=flo, start=False, stop=False)
                        nc.tensor.matmul(out=po, lhsT=wlo, rhs=fhi, start=False, stop=True)
                bhi, blo = btiles[k]
                src = pst[:, : gs * PPB].rearrange("p (g bb) -> p g bb", bb=PPB)
                dhi = bhi[:].rearrange("p (bb i) -> p i bb", i=F)[:, i0 : i0 + gs, :]
                dlo = blo[:].rearrange("p (bb i) -> p i bb", i=F)[:, i0 : i0 + gs, :]
                # hi = bf16(psum) on ACT, then lo = bf16(psum - hi) on DVE
                nc.scalar.copy(out=dhi, in_=src)
                nc.vector.tensor_sub(out=dlo, in0=src, in1=dhi)

            def e2_stage_group(k, m):
                """einsum2 for pairs [m, m+STG) of block k + staged out-DMA.

                i-chunks are stride-2 interleaved (ci = i%2) so out partition p
                owns rows i=2p, 2p+1 of each out[b]."""
                nonlocal dma_i
                ftile = ftiles[k]
                bhi, blo = btiles[k]
                bh4 = bhi[:].rearrange("p (bb i2 ci) -> p bb ci i2", i2=100, ci=2)
                bl4 = blo[:].rearrange("p (bb i2 ci) -> p bb ci i2", i2=100, ci=2)
                stage = stpool.tile([128, STG * 4 * F], f32)
                for u in range(STG):
                    bb = m + u
                    psA = ps2pool.tile([128, 512], f32)
                    psB = ps2pool.tile([128, 512], f32)
                    for ci in (0, 1):
                        for p, pst2 in ((0, psA), (1, psB)):
                            pr = slice(p * 64, (p + 1) * 64)
                            po = pst2[0:100, ci * 256 : ci * 256 + F]
                            lh = bh4[pr, bb, ci, :]
                            ll = bl4[pr, bb, ci, :]
                            fhi = ftile[pr, bb * 2 * F : bb * 2 * F + F]
                            flo = ftile[pr, bb * 2 * F + F : bb * 2 * F + 2 * F]
                            nc.tensor.matmul(out=po, lhsT=lh, rhs=fhi, start=True, stop=False)
                            nc.tensor.matmul(out=po, lhsT=lh, rhs=flo, start=False, stop=False)
                            nc.tensor.matmul(out=po, lhsT=ll, rhs=fhi, start=False, stop=True)
                    # both parity copies run concurrently on different engines
                    for p, pst2, eng in (
                        (0, psA, nc.vector.tensor_copy),
                        (1, psB, nc.scalar.copy),
                    ):
                        src = pst2[0:100].rearrange("q (ci j) -> q ci j", ci=2)[
                            :, :, 0:F
                        ]
                        b_loc = u * 2 + p
                        dst = stage[
                            0:100, b_loc * 2 * F : (b_loc + 1) * 2 * F
                        ].rearrange("q (ci j) -> q ci j", ci=2)
                        eng(out=dst, in_=src)
                b0 = k * 2 * PPB + 2 * m
                dma_eng = nc.sync if dma_i % 2 == 0 else nc.scalar
                dma_i += 1
                dma_eng.dma_start(
                    out=out_v[:, b0 : b0 + 2 * STG, :, :],
                    in_=stage[0:100, :].rearrange(
                        "p (b ci j) -> p b ci j", ci=2, j=F
                    ),
                )

            # Software-pipelined schedule: einsum1 of block k is emitted
            # interleaved with einsum2 of block k-1 so the PE instruction
            # stream stays dense (keeps the HAM clock-gate warm) and e2's
            # dependency latency hides under e1's matmul work.
            load_block(0)
            n_groups = (F + IGRP - 1) // IGRP
            e2_ms = list(range(0, PPB, STG))
            for k in range(BLOCKS + 1):
                if k + 1 < BLOCKS:
                    load_block(k + 1)
                g_i = 0
                m_i = 0
                while (k < BLOCKS and g_i < n_groups) or (k > 0 and m_i < len(e2_ms)):
                    if k < BLOCKS and g_i < n_groups:
                        e1_group(k, g_i * IGRP)
                        g_i += 1
                    if k > 0:
                        # spread e2 stage-groups of the previous block evenly
                        # across this block's e1 groups
                        target = (
                            len(e2_ms)
                            if k == BLOCKS or g_i >= n_groups
                            else (g_i * len(e2_ms)) // n_groups
                        )
                        while m_i < min(target, len(e2_ms)):
                            e2_stage_group(k - 1, e2_ms[m_i])
                            m_i += 1

    nc.compile()
    return nc


class _Runner:
    """Builds the program once and keeps a reusable sharded jit."""

    def __init__(self):
        self.nc = _build_program()
        import jax
        from jax.sharding import Mesh, PartitionSpec
        from jax.experimental.shard_map import shard_map
        from concourse import mybir
        from concourse import bass2jax

        bass2jax.install_neuronx_cc_hook()
        nc = self.nc

        partition_name = (
            nc.partition_id_tensor.name if nc.partition_id_tensor else None
        )
        in_names, out_names, out_avals, zero_outs = [], [], [], []
        for alloc in nc.m.functions[0].allocations:
            if not isinstance(alloc, mybir.MemoryLocationSet):
                continue
            name = alloc.memorylocations[0].name
            if alloc.kind == "ExternalInput":
                if name != partition_name:
                    in_names.append(name)
            elif alloc.kind == "ExternalOutput":
                shape = tuple(alloc.tensor_shape)
                dtype = mybir.dt.np(alloc.dtype)
                out_names.append(name)
                out_avals.append(jax.core.ShapedArray(shape, dtype))
                zero_outs.append(np.zeros(shape, dtype))
        self.in_names = list(in_names)
        self.out_names = out_names
        self.out_avals = out_avals
        self.zero_outs = zero_outs
        n_params = len(in_names)
        n_outs = len(out_avals)
        in_names_full = in_names + out_names
        if partition_name is not None:
            in_names_full.append(partition_name)
        donate = tuple(range(n_params, n_params + n_outs))

        def _body(*args):
            operands = list(args)
            if partition_name is not None:
                operands.append(bass2jax.partition_id_tensor())
            outs = bass2jax._bass_exec_p.bind(
                *operands,
                out_avals=tuple(out_avals),
                in_names=tuple(in_names_full),
                out_names=tuple(out_names),
                lowering_input_output_aliases=(),
                sim_require_finite=True,
                sim_require_nnan=True,
                nc=nc,
            )
            return tuple(outs)

        devices = jax.devices()[:NCORES]
        mesh = Mesh(np.asarray(devices), ("core",))
        in_specs = (PartitionSpec("core"),) * (n_params + n_outs)
        out_specs = (PartitionSpec("core"),) * n_outs
        self.sharded = jax.jit(
            shard_map(
                _body,
                mesh=mesh,
                in_specs=in_specs,
                out_specs=out_specs,
                check_rep=False,
            ),
            donate_argnums=donate,
            keep_unused=True,
        )

    def run(self, concat_inputs):
        """concat_inputs: dict name -> (8*shape0, ...) array."""
        args = [concat_inputs[n] for n in self.in_names]
        zeros = [
            np.zeros((NCORES * z.shape[0], *z.shape[1:]), z.dtype)
            for z in self.zero_outs
        ]
        outs = self.sharded(*args, *zeros)
        return {n: np.asarray(outs[i]) for i, n in enumerate(self.out_names)}


def _get_runner():
    global _RUNNER
    if _RUNNER is None:
        _RUNNER = _Runner()
    return _RUNNER


def pack_inputs(feature, weight):
    """Host-side packing: returns dict of concatenated per-core inputs.

    Inputs are split into bf16 hi/lo pairs (x = hi + lo) for the on-chip
    split-precision matmuls.
    """
    import ml_dtypes

    bf16 = ml_dtypes.bfloat16
    feature = np.ascontiguousarray(np.asarray(feature, dtype=np.float32))
    weight = np.ascontiguousarray(np.asarray(weight, dtype=np.float32))

    fh = feature.astype(bf16)
    fl = (feature - fh.astype(np.float32)).astype(bf16)
    # fpk[core][par*64+e, bb*(2F) + hl*F + i] = f_hl[core*BLOC + 2*bb + par, i, e]
    fs = np.stack([fh, fl], axis=0).reshape(2, NCORES, NPAIR, 2, F, E)
    fpk = np.ascontiguousarray(fs.transpose(1, 3, 5, 2, 0, 4)).reshape(
        NCORES * 128, NPAIR * 2 * F
    )

    wh = weight.astype(bf16)
    wl = (weight - wh.astype(np.float32)).astype(bf16)
    # wpk[par*64+e, i*(2E) + hl*E + d] = w_hl[i, e, d]
    ws = np.stack([wh, wl], axis=0)  # (2, F, E, E) [hl, i, e, d]
    wv = np.ascontiguousarray(ws.transpose(2, 1, 0, 3)).reshape(E, F * 2 * E)
    wpk_one = np.concatenate([wv, wv], axis=0)  # (128, F*2E)
    wpk = np.tile(wpk_one, (NCORES, 1))
    return {"fpk": fpk, "wpk": wpk}


def kernel(feature, weight):
    r = _get_runner()
    ins = pack_inputs(feature, weight)
    outs = r.run(ins)
    return unpack_output(outs["out"])


def unpack_output(out_dev):
    """out_dev: (8*100, BLOC, 2, F) device layout -> (B, F, F)."""
    o = out_dev.reshape(NCORES, 100, BLOC, 2, F)
    # out[core, b, 2p+ci, j] = o[core, p, b, ci, j]
    return np.ascontiguousarray(o.transpose(0, 2, 1, 3, 4)).reshape(B, F, F)


if __name__ == "__main__":
    rng = np.random.default_rng(0)
    feature = rng.standard_normal((B, F, E), dtype=np.float32)
    weight = (0.01 * rng.standard_normal((F, E, E))).astype(np.float32)
    got = kernel(feature, weight)
    bil = np.einsum("bie,ied->bid", feature.astype(np.float64), weight.astype(np.float64))
    ref = np.einsum("bid,bjd->bij", bil, feature.astype(np.float64))
    err = np.abs(got - ref)
    denom = np.abs(ref).max()
    print("max abs err:", err.max(), "rel(scale):", err.max() / denom)
    l2 = np.linalg.norm((got - ref).ravel()) / np.linalg.norm(ref.ravel())
    print("L2 rel:", l2)


# revision 31
# speedup vs baseline: 1.1703x; 1.1703x over previous
"""Trainium2 Bass kernel for nn_BiLinearDotLayer.

Computes, for feature (B,F,E)=(2048,200,64) f32 and weight (F,E,E):
    bilinear[b,i,d] = sum_e feature[b,i,e] * weight[i,e,d]
    out[b,i,j]      = sum_d bilinear[b,i,d] * feature[b,j,d]

Strategy (8 NeuronCores, data-parallel over batch):
  - Each core handles 256 batches; weight replicated.
  - Host pre-transposes feature to featT[e, b, i] and packs even/odd
    batches into SBUF partition halves (p*64+e) so all on-chip tiles use
    128 partitions (full DMA width) and the two batch parities run
    concurrently on the PE array's row-strips (K=64 each).
  - Both einsums run fully on-chip per block of 128 batches; only the
    feature shard and weight are read and only the final (256,200,200)
    f32 output is written per core.
  - einsum1 runs in exact fp32 (2-pass); einsum2 runs in fp16
    (1 cycle/row, ~5e-4 rel error). The fp16 feature copy is produced
    on-chip by DVE/ACT casts so no extra HBM traffic is added.
"""

import os
import sys

for _p in ("/opt/trn_rl_repo", "/root/.axon_site/_ro/trn_rl_repo"):
    if os.path.isdir(_p) and _p not in sys.path:
        sys.path.insert(0, _p)

import numpy as np

B, F, E = 2048, 200, 64
NCORES = 8
BLOC = B // NCORES            # 256 batches per core
NPAIR = BLOC // 2             # 128 even/odd batch pairs per core
BLOCKS = 2
PPB = NPAIR // BLOCKS         # 64 pairs (128 batches) per block
IGRP = 8                      # einsum1 i's per PSUM group (1 bank)
STG = 2                       # pairs per staged out-DMA (4 batches, 640KB)
NCAST = 8                     # chunks per block for the f32->fp16 feature cast

_RUNNER = None


def _build_program():
    import concourse.tile as tile
    from concourse import bacc, mybir

    f32 = mybir.dt.float32
    fp16 = mybir.dt.float16
    nc = bacc.Bacc("TRN2", target_bir_lowering=False, debug=False)

    # feature packed as featT: fpk[p*64+e, bb*F+i] (fp32)
    fpk = nc.dram_tensor("fpk", [128, NPAIR * F], f32, kind="ExternalInput")
    # weight packed as wpk[p*64+e, i*64+d] (fp32)
    wpk = nc.dram_tensor("wpk", [128, F * E], f32, kind="ExternalInput")
    # Device-friendly output layout: out_dev[p, b, ci, j] = out[b, 2p+ci, j].
    # Each partition's slice is contiguous in DRAM, so out-DMA descriptors are
    # one 12.8KB run per partition per stage group (vs 800B interleaved runs).
    # The host un-permutes afterwards.
    out = nc.dram_tensor("out", [100, BLOC, 2, F], f32, kind="ExternalOutput")
    out_v = out.ap()

    with tile.TileContext(nc) as tc:
        with (
            tc.tile_pool(name="wpool", bufs=3) as wpool,
            tc.tile_pool(name="fpool", bufs=2) as fpool,
            tc.tile_pool(name="bpool", bufs=1) as bpool,
            tc.tile_pool(name="stpool", bufs=3) as stpool,
            tc.tile_pool(name="ps1", bufs=2, space="PSUM") as ps1pool,
            tc.tile_pool(name="ps2", bufs=3, space="PSUM") as ps2pool,
        ):
            ftiles = [None] * BLOCKS
            f16tiles = [None] * BLOCKS
            btiles = [None] * BLOCKS

            def load_block(k):
                ftiles[k] = fpool.tile([128, PPB * F], f32, name="ftile", tag="ftile")
                nc.sync.dma_start(
                    out=ftiles[k][:],
                    in_=fpk.ap()[:, k * PPB * F : (k + 1) * PPB * F],
                )
                # fp16 copy for einsum2, cast in chunks on DVE+ACT
                f16tiles[k] = fpool.tile(
                    [128, PPB * F], fp16, name="f16tile", tag="f16tile"
                )
                cw = PPB * F // NCAST
                for c in range(NCAST):
                    sl = slice(c * cw, (c + 1) * cw)
                    if c % 2 == 0:
                        nc.vector.tensor_copy(out=f16tiles[k][:, sl], in_=ftiles[k][:, sl])
                    else:
                        nc.scalar.copy(out=f16tiles[k][:, sl], in_=ftiles[k][:, sl])
                btiles[k] = bpool.tile([128, PPB * F], fp16, name="btile", tag="btile")

            dma_i = 0  # alternates out-DMAs between sync and scalar rings
            cpy = 0

            def e1_group(k, i0):
                """einsum1 for i in [i0, i0+gs) of block k (exact fp32)."""
                nonlocal cpy
                gs = min(IGRP, F - i0)
                f3 = ftiles[k][:].rearrange("p (bb i) -> p bb i", i=F)
                wseg = wpool.tile([128, IGRP * E], f32, name="wseg", tag="wseg")
                nc.sync.dma_start(
                    out=wseg[:, : gs * E], in_=wpk.ap()[:, i0 * E : (i0 + gs) * E]
                )
                pst = ps1pool.tile([128, IGRP * PPB], f32)
                for g in range(gs):
                    i = i0 + g
                    for p in (0, 1):
                        pr = slice(p * 64, (p + 1) * 64)
                        nc.tensor.matmul(
                            out=pst[pr, g * PPB : (g + 1) * PPB],
                            lhsT=wseg[pr, g * E : (g + 1) * E],
                            rhs=f3[pr, :, i],
                            start=True,
                            stop=True,
                        )
                src = pst[:, : gs * PPB].rearrange("p (g bb) -> p g bb", bb=PPB)
                dst = btiles[k][:].rearrange("p (bb i) -> p i bb", i=F)[
                    :, i0 : i0 + gs, :
                ]
                # cast fp32 psum -> fp16 bilinear tile
                if cpy % 2 == 0:
                    nc.vector.tensor_copy(out=dst, in_=src)
                else:
                    nc.scalar.copy(out=dst, in_=src)
                cpy += 1

            def e2_stage_group(k, m):
                """einsum2 for pairs [m, m+STG) of block k + staged out-DMA.

                i-chunks are stride-2 interleaved (ci = i%2) so out partition p
                owns rows i=2p, 2p+1 of each out[b]."""
                nonlocal dma_i
                f16t = f16tiles[k]
                btile = btiles[k]
                bt4 = btile[:].rearrange("p (bb i2 ci) -> p bb ci i2", i2=100, ci=2)
                stage = stpool.tile([128, STG * 4 * F], f32)
                for u in range(STG):
                    bb = m + u
                    psA = ps2pool.tile([128, 512], f32)
                    psB = ps2pool.tile([128, 512], f32)
                    for ci in (0, 1):
                        for p, pst2 in ((0, psA), (1, psB)):
                            pr = slice(p * 64, (p + 1) * 64)
                            nc.tensor.matmul(
                                out=pst2[0:100, ci * 256 : ci * 256 + F],
                                lhsT=bt4[pr, bb, ci, :],
                                rhs=f16t[pr, bb * F : (bb + 1) * F],
                                start=True,
                                stop=True,
                            )
                    # both parity copies run concurrently on different engines
                    for p, pst2, eng in (
                        (0, psA, nc.vector.tensor_copy),
                        (1, psB, nc.scalar.copy),
                    ):
                        src = pst2[0:100].rearrange("q (ci j) -> q ci j", ci=2)[
                            :, :, 0:F
                        ]
                        b_loc = u * 2 + p
                        dst = stage[
                            0:100, b_loc * 2 * F : (b_loc + 1) * 2 * F
                        ].rearrange("q (ci j) -> q ci j", ci=2)
                        eng(out=dst, in_=src)
                b0 = k * 2 * PPB + 2 * m
                dma_eng = nc.sync if dma_i % 2 == 0 else nc.scalar
                dma_i += 1
                dma_eng.dma_start(
                    out=out_v[:, b0 : b0 + 2 * STG, :, :],
                    in_=stage[0:100, :].rearrange(
                        "p (b ci j) -> p b ci j", ci=2, j=F
                    ),
                )

            # Software-pipelined schedule: einsum1 of block k is emitted
            # interleaved with einsum2 of block k-1 so the PE instruction
            # stream stays dense (keeps the HAM clock-gate warm) and e2's
            # dependency latency hides under e1's matmul work.
            load_block(0)
            n_groups = (F + IGRP - 1) // IGRP
            e2_ms = list(range(0, PPB, STG))
            for k in range(BLOCKS + 1):
                if k + 1 < BLOCKS:
                    load_block(k + 1)
                g_i = 0
                m_i = 0
                while (k < BLOCKS and g_i < n_groups) or (k > 0 and m_i < len(e2_ms)):
                    if k < BLOCKS and g_i < n_groups:
                        e1_group(k, g_i * IGRP)
                        g_i += 1
                    if k > 0:
                        # spread e2 stage-groups of the previous block evenly
                        # across this block's e1 groups
                        target = (
                            len(e2_ms)
                            if k == BLOCKS or g_i >= n_groups
                            else (g_i * len(e2_ms)) // n_groups
                        )
                        while m_i < min(target, len(e2_ms)):
                            e2_stage_group(k - 1, e2_ms[m_i])
                            m_i += 1

    nc.compile()
    return nc


class _Runner:
    """Builds the program once and keeps a reusable sharded jit."""

    def __init__(self):
        self.nc = _build_program()
        import jax
        from jax.sharding import Mesh, PartitionSpec
        from jax.experimental.shard_map import shard_map
        from concourse import mybir
        from concourse import bass2jax

        bass2jax.install_neuronx_cc_hook()
        nc = self.nc

        partition_name = (
            nc.partition_id_tensor.name if nc.partition_id_tensor else None
        )
        in_names, out_names, out_avals, zero_outs = [], [], [], []
        for alloc in nc.m.functions[0].allocations:
            if not isinstance(alloc, mybir.MemoryLocationSet):
                continue
            name = alloc.memorylocations[0].name
            if alloc.kind == "ExternalInput":
                if name != partition_name:
                    in_names.append(name)
            elif alloc.kind == "ExternalOutput":
                shape = tuple(alloc.tensor_shape)
                dtype = mybir.dt.np(alloc.dtype)
                out_names.append(name)
                out_avals.append(jax.core.ShapedArray(shape, dtype))
                zero_outs.append(np.zeros(shape, dtype))
        self.in_names = list(in_names)
        self.out_names = out_names
        self.out_avals = out_avals
        self.zero_outs = zero_outs
        n_params = len(in_names)
        n_outs = len(out_avals)
        in_names_full = in_names + out_names
        if partition_name is not None:
            in_names_full.append(partition_name)
        donate = tuple(range(n_params, n_params + n_outs))

        def _body(*args):
            operands = list(args)
            if partition_name is not None:
                operands.append(bass2jax.partition_id_tensor())
            outs = bass2jax._bass_exec_p.bind(
                *operands,
                out_avals=tuple(out_avals),
                in_names=tuple(in_names_full),
                out_names=tuple(out_names),
                lowering_input_output_aliases=(),
                sim_require_finite=True,
                sim_require_nnan=True,
                nc=nc,
            )
            return tuple(outs)

        devices = jax.devices()[:NCORES]
        mesh = Mesh(np.asarray(devices), ("core",))
        in_specs = (PartitionSpec("core"),) * (n_params + n_outs)
        out_specs = (PartitionSpec("core"),) * n_outs
        self.sharded = jax.jit(
            shard_map(
                _body,
                mesh=mesh,
                in_specs=in_specs,
                out_specs=out_specs,
                check_rep=False,
            ),
            donate_argnums=donate,
            keep_unused=True,
        )

    def run(self, concat_inputs):
        """concat_inputs: dict name -> (8*shape0, ...) array."""
        args = [concat_inputs[n] for n in self.in_names]
        zeros = [
            np.zeros((NCORES * z.shape[0], *z.shape[1:]), z.dtype)
            for z in self.zero_outs
        ]
        outs = self.sharded(*args, *zeros)
        return {n: np.asarray(outs[i]) for i, n in enumerate(self.out_names)}


def _get_runner():
    global _RUNNER
    if _RUNNER is None:
        _RUNNER = _Runner()
    return _RUNNER


def pack_inputs(feature, weight):
    """Host-side packing: returns dict of concatenated per-core inputs."""
    feature = np.ascontiguousarray(np.asarray(feature, dtype=np.float32))
    weight = np.ascontiguousarray(np.asarray(weight, dtype=np.float32))
    # featT pack: fpk[core][p*64+e, bb*F+i] = feature[core*BLOC + 2*bb + p, i, e]
    ft = feature.reshape(NCORES, NPAIR, 2, F, E)  # [core, bb, p, i, e]
    fpk = np.ascontiguousarray(ft.transpose(0, 2, 4, 1, 3)).reshape(
        NCORES * 128, NPAIR * F
    )
    wt = np.ascontiguousarray(weight.transpose(1, 0, 2)).reshape(E, F * E)
    wpk_one = np.concatenate([wt, wt], axis=0)  # (128, F*E)
    wpk = np.tile(wpk_one, (NCORES, 1))
    return {"fpk": fpk, "wpk": wpk}


def kernel(feature, weight):
    r = _get_runner()
    ins = pack_inputs(feature, weight)
    outs = r.run(ins)
    return unpack_output(outs["out"])


def unpack_output(out_dev):
    """out_dev: (8*100, BLOC, 2, F) device layout -> (B, F, F)."""
    o = out_dev.reshape(NCORES, 100, BLOC, 2, F)
    # out[core, b, 2p+ci, j] = o[core, p, b, ci, j]
    return np.ascontiguousarray(o.transpose(0, 2, 1, 3, 4)).reshape(B, F, F)


if __name__ == "__main__":
    rng = np.random.default_rng(0)
    feature = rng.standard_normal((B, F, E), dtype=np.float32)
    weight = (0.01 * rng.standard_normal((F, E, E))).astype(np.float32)
    got = kernel(feature, weight)
    bil = np.einsum("bie,ied->bid", feature.astype(np.float64), weight.astype(np.float64))
    ref = np.einsum("bid,bjd->bij", bil, feature.astype(np.float64))
    err = np.abs(got - ref)
    denom = np.abs(ref).max()
    print("max abs err:", err.max(), "rel(scale):", err.max() / denom)
    l2 = np.linalg.norm((got - ref).ravel()) / np.linalg.norm(ref.ravel())
    print("L2 rel:", l2)


# revision 32
# speedup vs baseline: 1.4475x; 1.2369x over previous
"""Trainium2 Bass kernel for nn_BiLinearDotLayer.

Computes, for feature (B,F,E)=(2048,200,64) f32 and weight (F,E,E):
    bilinear[b,i,d] = sum_e feature[b,i,e] * weight[i,e,d]
    out[b,i,j]      = sum_d bilinear[b,i,d] * feature[b,j,d]

Strategy (8 NeuronCores, data-parallel over batch):
  - Each core handles 256 batches; weight replicated.
  - Host pre-transposes feature to featT[e, b, i] (fp16) and packs
    even/odd batches into SBUF partition halves (p*64+e) so all tiles
    use 128 partitions (full DMA width) and the two batch parities run
    concurrently on the PE array's row-strips (K=64 each).
  - Single resident block: the whole 256-batch shard + weights live in
    SBUF; both einsums run fully on-chip (fp16 operands, fp32 PSUM
    accumulation, ~5e-4 rel error); only the fp16 feature (6.5MB),
    fp16 weights (3.3MB) are read and the f32 output (41MB) written.
  - Output is written in a partition-contiguous device layout and
    un-permuted on the host.
"""

import os
import sys

for _p in ("/opt/trn_rl_repo", "/root/.axon_site/_ro/trn_rl_repo"):
    if os.path.isdir(_p) and _p not in sys.path:
        sys.path.insert(0, _p)

import numpy as np

B, F, E = 2048, 200, 64
NCORES = 8
BLOC = B // NCORES            # 256 batches per core
NPAIR = BLOC // 2             # 128 even/odd batch pairs per core
IGRP = 4                      # einsum1 i's per PSUM group (4*128 = 1 bank)
STG = 4                       # pairs per staged out-DMA (8 batches, 1.28MB)

_RUNNER = None


def _build_program():
    import concourse.tile as tile
    from concourse import bacc, mybir

    f32 = mybir.dt.float32
    fp16 = mybir.dt.float16
    nc = bacc.Bacc("TRN2", target_bir_lowering=False, debug=False)

    # feature packed as featT: fpk[p*64+e, bb*F+i] (fp16)
    fpk = nc.dram_tensor("fpk", [128, NPAIR * F], fp16, kind="ExternalInput")
    # weight packed as wpk[p*64+e, i*64+d] (fp16)
    wpk = nc.dram_tensor("wpk", [128, F * E], fp16, kind="ExternalInput")
    # Device-friendly output layout: out_dev[p, b, ci, j] = out[b, 2p+ci, j].
    # Each partition's slice is contiguous in DRAM; host un-permutes.
    out = nc.dram_tensor("out", [100, BLOC, 2, F], f32, kind="ExternalOutput")
    out_v = out.ap()

    with tile.TileContext(nc) as tc:
        with (
            tc.tile_pool(name="wpool", bufs=1) as wpool,
            tc.tile_pool(name="fpool", bufs=1) as fpool,
            tc.tile_pool(name="bpool", bufs=1) as bpool,
            tc.tile_pool(name="stpool", bufs=4) as stpool,
            tc.tile_pool(name="ps1", bufs=3, space="PSUM") as ps1pool,
            tc.tile_pool(name="ps2", bufs=2, space="PSUM") as ps2pool,
        ):
            # whole-shard resident tiles
            ftile = fpool.tile([128, NPAIR * F], fp16, name="ftile", tag="ftile")
            # feature on sync ring, weights on scalar ring (parallel loads);
            # feature in two halves so einsum1 can start after the first.
            half = NPAIR * F // 2
            nc.sync.dma_start(out=ftile[:, :half], in_=fpk.ap()[:, :half])
            nc.sync.dma_start(out=ftile[:, half:], in_=fpk.ap()[:, half:])
            wtile = wpool.tile([128, F * E], fp16, name="wtile", tag="wtile")
            nc.scalar.dma_start(out=wtile[:], in_=wpk.ap())
            btile = bpool.tile([128, NPAIR * F], fp16, name="btile", tag="btile")

            f3 = ftile[:].rearrange("p (bb i) -> p bb i", i=F)
            b3w = btile[:].rearrange("p (bb i) -> p i bb", i=F)
            bt4 = btile[:].rearrange("p (bb i2 ci) -> p bb ci i2", i2=100, ci=2)

            # ---- einsum1: bilinearT[d, bb] per i, N = all 128 pairs ----
            cpy = 0
            for i0 in range(0, F, IGRP):
                gs = min(IGRP, F - i0)
                pst = ps1pool.tile([128, IGRP * NPAIR], f32)
                for g in range(gs):
                    i = i0 + g
                    for p in (0, 1):
                        pr = slice(p * 64, (p + 1) * 64)
                        nc.tensor.matmul(
                            out=pst[pr, g * NPAIR : (g + 1) * NPAIR],
                            lhsT=wtile[pr, i * E : (i + 1) * E],
                            rhs=f3[pr, :, i],
                            start=True,
                            stop=True,
                        )
                src = pst[:, : gs * NPAIR].rearrange("p (g bb) -> p g bb", bb=NPAIR)
                dst = b3w[:, i0 : i0 + gs, :]
                # cast fp32 psum -> fp16 bilinear tile
                if cpy % 2 == 0:
                    nc.vector.tensor_copy(out=dst, in_=src)
                else:
                    nc.scalar.copy(out=dst, in_=src)
                cpy += 1

            # ---- einsum2: out[b] = bilinear[b] @ feature[b].T ----
            # i-chunks are stride-2 interleaved (ci = i%2) so out partition p
            # owns rows i=2p, 2p+1 of each out[b].
            dma_i = 0
            for m in range(0, NPAIR, STG):
                stage = stpool.tile([128, STG * 4 * F], f32)
                for u in range(STG):
                    bb = m + u
                    ps2 = ps2pool.tile([128, 1024], f32)
                    for ci in (0, 1):
                        for p in (0, 1):
                            pr = slice(p * 64, (p + 1) * 64)
                            nc.tensor.matmul(
                                out=ps2[
                                    0:100,
                                    p * 512 + ci * 256 : p * 512 + ci * 256 + F,
                                ],
                                lhsT=bt4[pr, bb, ci, :],
                                rhs=ftile[pr, bb * F : (bb + 1) * F],
                                start=True,
                                stop=True,
                            )
                    # one copy per pair (both parities + both i-chunks)
                    src = ps2[0:100].rearrange("q (p ci j) -> q p ci j", p=2, ci=2)[
                        :, :, :, 0:F
                    ]
                    dst = stage[0:100, u * 4 * F : (u + 1) * 4 * F].rearrange(
                        "q (p ci j) -> q p ci j", p=2, ci=2
                    )
                    if cpy % 2 == 0:
                        nc.vector.tensor_copy(out=dst, in_=src)
                    else:
                        nc.scalar.copy(out=dst, in_=src)
                    cpy += 1
                b0 = 2 * m
                dma_eng = nc.sync if dma_i % 2 == 0 else nc.scalar
                dma_i += 1
                dma_eng.dma_start(
                    out=out_v[:, b0 : b0 + 2 * STG, :, :],
                    in_=stage[0:100, :].rearrange(
                        "p (b ci j) -> p b ci j", ci=2, j=F
                    ),
                )

    nc.compile()
    return nc


class _Runner:
    """Builds the program once and keeps a reusable sharded jit."""

    def __init__(self):
        self.nc = _build_program()
        import jax
        from jax.sharding import Mesh, PartitionSpec
        from jax.experimental.shard_map import shard_map
        from concourse import mybir
        from concourse import bass2jax

        bass2jax.install_neuronx_cc_hook()
        nc = self.nc

        partition_name = (
            nc.partition_id_tensor.name if nc.partition_id_tensor else None
        )
        in_names, out_names, out_avals, zero_outs = [], [], [], []
        for alloc in nc.m.functions[0].allocations:
            if not isinstance(alloc, mybir.MemoryLocationSet):
                continue
            name = alloc.memorylocations[0].name
            if alloc.kind == "ExternalInput":
                if name != partition_name:
                    in_names.append(name)
            elif alloc.kind == "ExternalOutput":
                shape = tuple(alloc.tensor_shape)
                dtype = mybir.dt.np(alloc.dtype)
                out_names.append(name)
                out_avals.append(jax.core.ShapedArray(shape, dtype))
                zero_outs.append(np.zeros(shape, dtype))
        self.in_names = list(in_names)
        self.out_names = out_names
        self.out_avals = out_avals
        self.zero_outs = zero_outs
        n_params = len(in_names)
        n_outs = len(out_avals)
        in_names_full = in_names + out_names
        if partition_name is not None:
            in_names_full.append(partition_name)
        donate = tuple(range(n_params, n_params + n_outs))

        def _body(*args):
            operands = list(args)
            if partition_name is not None:
                operands.append(bass2jax.partition_id_tensor())
            outs = bass2jax._bass_exec_p.bind(
                *operands,
                out_avals=tuple(out_avals),
                in_names=tuple(in_names_full),
                out_names=tuple(out_names),
                lowering_input_output_aliases=(),
                sim_require_finite=True,
                sim_require_nnan=True,
                nc=nc,
            )
            return tuple(outs)

        devices = jax.devices()[:NCORES]
        mesh = Mesh(np.asarray(devices), ("core",))
        in_specs = (PartitionSpec("core"),) * (n_params + n_outs)
        out_specs = (PartitionSpec("core"),) * n_outs
        self.sharded = jax.jit(
            shard_map(
                _body,
                mesh=mesh,
                in_specs=in_specs,
                out_specs=out_specs,
                check_rep=False,
            ),
            donate_argnums=donate,
            keep_unused=True,
        )

    def run(self, concat_inputs):
        """concat_inputs: dict name -> (8*shape0, ...) array."""
        args = [concat_inputs[n] for n in self.in_names]
        zeros = [
            np.zeros((NCORES * z.shape[0], *z.shape[1:]), z.dtype)
            for z in self.zero_outs
        ]
        outs = self.sharded(*args, *zeros)
        return {n: np.asarray(outs[i]) for i, n in enumerate(self.out_names)}


def _get_runner():
    global _RUNNER
    if _RUNNER is None:
        _RUNNER = _Runner()
    return _RUNNER


def pack_inputs(feature, weight):
    """Host-side packing: returns dict of concatenated per-core inputs."""
    feature = np.ascontiguousarray(np.asarray(feature, dtype=np.float32))
    weight = np.ascontiguousarray(np.asarray(weight, dtype=np.float32))
    # featT pack: fpk[core][p*64+e, bb*F+i] = feature[core*BLOC + 2*bb + p, i, e]
    ft = feature.reshape(NCORES, NPAIR, 2, F, E)  # [core, bb, p, i, e]
    fpk = (
        np.ascontiguousarray(ft.transpose(0, 2, 4, 1, 3))
        .reshape(NCORES * 128, NPAIR * F)
        .astype(np.float16)
    )
    wt = np.ascontiguousarray(weight.transpose(1, 0, 2)).reshape(E, F * E)
    wpk_one = np.concatenate([wt, wt], axis=0).astype(np.float16)  # (128, F*E)
    wpk = np.tile(wpk_one, (NCORES, 1))
    return {"fpk": fpk, "wpk": wpk}


def kernel(feature, weight):
    r = _get_runner()
    ins = pack_inputs(feature, weight)
    outs = r.run(ins)
    return unpack_output(outs["out"])


def unpack_output(out_dev):
    """out_dev: (8*100, BLOC, 2, F) device layout -> (B, F, F)."""
    o = out_dev.reshape(NCORES, 100, BLOC, 2, F)
    # out[core, b, 2p+ci, j] = o[core, p, b, ci, j]
    return np.ascontiguousarray(o.transpose(0, 2, 1, 3, 4)).reshape(B, F, F)


if __name__ == "__main__":
    rng = np.random.default_rng(0)
    feature = rng.standard_normal((B, F, E), dtype=np.float32)
    weight = (0.01 * rng.standard_normal((F, E, E))).astype(np.float32)
    got = kernel(feature, weight)
    bil = np.einsum(
        "bie,ied->bid", feature.astype(np.float64), weight.astype(np.float64)
    )
    ref = np.einsum("bid,bjd->bij", bil, feature.astype(np.float64))
    err = np.abs(got - ref)
    denom = np.abs(ref).max()
    print("max abs err:", err.max(), "rel(scale):", err.max() / denom)
    l2 = np.linalg.norm((got - ref).ravel()) / np.linalg.norm(ref.ravel())
    print("L2 rel:", l2)


# revision 34
# speedup vs baseline: 1.6079x; 1.1108x over previous
"""Trainium2 Bass kernel for nn_BiLinearDotLayer.

Computes, for feature (B,F,E)=(2048,200,64) f32 and weight (F,E,E):
    bilinear[b,i,d] = sum_e feature[b,i,e] * weight[i,e,d]
    out[b,i,j]      = sum_d bilinear[b,i,d] * feature[b,j,d]

Strategy (8 NeuronCores, data-parallel over batch):
  - Each core handles 256 batches; weight replicated.
  - Host pre-transposes feature to featT[e, b, i] (fp16) and packs
    even/odd batches into SBUF partition halves (p*64+e) so all tiles
    use 128 partitions (full DMA width) and the two batch parities run
    concurrently on the PE array's row-strips (K=64 each).
  - Single resident block: the whole 256-batch shard + weights live in
    SBUF; both einsums run fully on-chip (fp16 operands, fp32 PSUM
    accumulation, ~5e-4 rel error); only the fp16 feature (6.5MB),
    fp16 weights (3.3MB) are read and the f32 output (41MB) written.
  - Output is written in a partition-contiguous device layout and
    un-permuted on the host.
"""

import os
import sys

for _p in ("/opt/trn_rl_repo", "/root/.axon_site/_ro/trn_rl_repo"):
    if os.path.isdir(_p) and _p not in sys.path:
        sys.path.insert(0, _p)

import numpy as np

B, F, E = 2048, 200, 64
NCORES = 8
BLOC = B // NCORES            # 256 batches per core
NPAIR = BLOC // 2             # 128 even/odd batch pairs per core
IGRP = 8                      # einsum1 i's per PSUM group (8*64 = 1 bank)
STG = 4                       # pairs per staged out-DMA (8 batches, 1.28MB)

_RUNNER = None


def _build_program():
    import concourse.tile as tile
    from concourse import bacc, mybir

    f32 = mybir.dt.float32
    fp16 = mybir.dt.float16
    nc = bacc.Bacc("TRN2", target_bir_lowering=False, debug=False)

    # feature packed as featT: fpk[p*64+e, bb*F+i] (fp16)
    fpk = nc.dram_tensor("fpk", [128, NPAIR * F], fp16, kind="ExternalInput")
    # weight packed as wpk[p*64+e, i*64+d] (fp16)
    wpk = nc.dram_tensor("wpk", [128, F * E], fp16, kind="ExternalInput")
    # Device-friendly output layout: out_dev[p, b, ci, j] = out[b, 2p+ci, j].
    # Each partition's slice is contiguous in DRAM; host un-permutes.
    out = nc.dram_tensor("out", [100, BLOC, 2, F], f32, kind="ExternalOutput")
    out_v = out.ap()

    HP = NPAIR // 2  # pairs per half-shard

    with tile.TileContext(nc) as tc:
        with (
            tc.tile_pool(name="wpool", bufs=1) as wpool,
            tc.tile_pool(name="fpool", bufs=1) as fpool,
            tc.tile_pool(name="bpool", bufs=1) as bpool,
            tc.tile_pool(name="stpool", bufs=4) as stpool,
            tc.tile_pool(name="ps1", bufs=2, space="PSUM") as ps1pool,
            tc.tile_pool(name="ps2", bufs=3, space="PSUM") as ps2pool,
        ):
            # whole-shard resident tiles
            ftile = fpool.tile([128, NPAIR * F], fp16, name="ftile", tag="ftile")
            # feature halves on sync ring (einsum1 of half 0 starts after the
            # first one); weights on scalar ring in parallel.
            half = HP * F
            nc.sync.dma_start(out=ftile[:, :half], in_=fpk.ap()[:, :half])
            nc.sync.dma_start(out=ftile[:, half:], in_=fpk.ap()[:, half:])
            wtile = wpool.tile([128, F * E], fp16, name="wtile", tag="wtile")
            nc.scalar.dma_start(out=wtile[:], in_=wpk.ap())
            # bilinear, i-major: btile[p*64+d, i*NPAIR + bb]
            btile = bpool.tile([128, NPAIR * F], fp16, name="btile", tag="btile")

            f3 = ftile[:].rearrange("p (bb i) -> p bb i", i=F)
            b3i = btile[:].rearrange("p (i bb) -> p i bb", bb=NPAIR)
            # lhsT view for einsum2: (p, ci, i2, bb) with i = 2*i2 + ci
            bt4 = btile[:].rearrange("p (i2 ci bb) -> p ci i2 bb", ci=2, bb=NPAIR)

            cpy = 0
            dma_i = 0

            def e1_group(h, i0):
                """einsum1 for i in [i0,i0+IGRP) x pairs of half h (N=HP)."""
                nonlocal cpy
                gs = min(IGRP, F - i0)
                pst = ps1pool.tile([128, IGRP * HP], f32, name="pst", tag="pst")
                for g in range(gs):
                    i = i0 + g
                    for p in (0, 1):
                        pr = slice(p * 64, (p + 1) * 64)
                        nc.tensor.matmul(
                            out=pst[pr, g * HP : (g + 1) * HP],
                            lhsT=wtile[pr, i * E : (i + 1) * E],
                            rhs=f3[pr, h * HP : (h + 1) * HP, i],
                            start=True,
                            stop=True,
                        )
                src = pst[:, : gs * HP].rearrange("p (g bb) -> p g bb", bb=HP)
                dst = b3i[:, i0 : i0 + gs, h * HP : (h + 1) * HP]
                # cast fp32 psum -> fp16 bilinear tile
                if cpy % 2 == 0:
                    nc.vector.tensor_copy(out=dst, in_=src)
                else:
                    nc.scalar.copy(out=dst, in_=src)
                cpy += 1

            def e2_stage_group(m):
                """einsum2 for pairs [m, m+STG) + staged out-DMA.

                i-chunks are stride-2 interleaved (ci = i%2) so out partition
                p owns rows i=2p, 2p+1 of each out[b]."""
                nonlocal dma_i
                stage = stpool.tile([128, STG * 4 * F], f32, name="stage", tag="stage")
                for u in range(STG):
                    bb = m + u
                    psA = ps2pool.tile([128, 512], f32, name="psA", tag="psA")
                    psB = ps2pool.tile([128, 512], f32, name="psB", tag="psB")
                    for ci in (0, 1):
                        for p, pst2 in ((0, psA), (1, psB)):
                            pr = slice(p * 64, (p + 1) * 64)
                            nc.tensor.matmul(
                                out=pst2[0:100, ci * 256 : ci * 256 + F],
                                lhsT=bt4[pr, ci, :, bb],
                                rhs=ftile[pr, bb * F : (bb + 1) * F],
                                start=True,
                                stop=True,
                            )
                    # parity copies run concurrently on different engines
                    for p, pst2, eng in (
                        (0, psA, nc.vector.tensor_copy),
                        (1, psB, nc.scalar.copy),
                    ):
                        src = pst2[0:100].rearrange("q (ci j) -> q ci j", ci=2)[
                            :, :, 0:F
                        ]
                        b_loc = u * 2 + p
                        dst = stage[
                            0:100, b_loc * 2 * F : (b_loc + 1) * 2 * F
                        ].rearrange("q (ci j) -> q ci j", ci=2)
                        eng(out=dst, in_=src)
                b0 = 2 * m
                dma_eng = nc.sync if dma_i % 2 == 0 else nc.scalar
                dma_i += 1
                dma_eng.dma_start(
                    out=out_v[:, b0 : b0 + 2 * STG, :, :],
                    in_=stage[0:100, :].rearrange(
                        "p (b ci j) -> p b ci j", ci=2, j=F
                    ),
                )

            # Schedule: e1(H0); then e1(H1) interleaved with e2(H0) so the PE
            # stream stays dense (HAM warm) and e2's chain latency hides under
            # e1's matmuls; then e2(H1).
            n_groups = (F + IGRP - 1) // IGRP
            for i0 in range(0, F, IGRP):
                e1_group(0, i0)
            h0_ms = list(range(0, HP, STG))
            g_list = list(range(0, F, IGRP))
            m_i = 0
            for gi, i0 in enumerate(g_list):
                e1_group(1, i0)
                target = ((gi + 1) * len(h0_ms)) // len(g_list)
                while m_i < target:
                    e2_stage_group(h0_ms[m_i])
                    m_i += 1
            for m in range(HP, NPAIR, STG):
                e2_stage_group(m)

    nc.compile()
    return nc


class _Runner:
    """Builds the program once and keeps a reusable sharded jit."""

    def __init__(self):
        self.nc = _build_program()
        import jax
        from jax.sharding import Mesh, PartitionSpec
        from jax.experimental.shard_map import shard_map
        from concourse import mybir
        from concourse import bass2jax

        bass2jax.install_neuronx_cc_hook()
        nc = self.nc

        partition_name = (
            nc.partition_id_tensor.name if nc.partition_id_tensor else None
        )
        in_names, out_names, out_avals, zero_outs = [], [], [], []
        for alloc in nc.m.functions[0].allocations:
            if not isinstance(alloc, mybir.MemoryLocationSet):
                continue
            name = alloc.memorylocations[0].name
            if alloc.kind == "ExternalInput":
                if name != partition_name:
                    in_names.append(name)
            elif alloc.kind == "ExternalOutput":
                shape = tuple(alloc.tensor_shape)
                dtype = mybir.dt.np(alloc.dtype)
                out_names.append(name)
                out_avals.append(jax.core.ShapedArray(shape, dtype))
                zero_outs.append(np.zeros(shape, dtype))
        self.in_names = list(in_names)
        self.out_names = out_names
        self.out_avals = out_avals
        self.zero_outs = zero_outs
        n_params = len(in_names)
        n_outs = len(out_avals)
        in_names_full = in_names + out_names
        if partition_name is not None:
            in_names_full.append(partition_name)
        donate = tuple(range(n_params, n_params + n_outs))

        def _body(*args):
            operands = list(args)
            if partition_name is not None:
                operands.append(bass2jax.partition_id_tensor())
            outs = bass2jax._bass_exec_p.bind(
                *operands,
                out_avals=tuple(out_avals),
                in_names=tuple(in_names_full),
                out_names=tuple(out_names),
                lowering_input_output_aliases=(),
                sim_require_finite=True,
                sim_require_nnan=True,
                nc=nc,
            )
            return tuple(outs)

        devices = jax.devices()[:NCORES]
        mesh = Mesh(np.asarray(devices), ("core",))
        in_specs = (PartitionSpec("core"),) * (n_params + n_outs)
        out_specs = (PartitionSpec("core"),) * n_outs
        self.sharded = jax.jit(
            shard_map(
                _body,
                mesh=mesh,
                in_specs=in_specs,
                out_specs=out_specs,
                check_rep=False,
            ),
            donate_argnums=donate,
            keep_unused=True,
        )

    def run(self, concat_inputs):
        """concat_inputs: dict name -> (8*shape0, ...) array."""
        args = [concat_inputs[n] for n in self.in_names]
        zeros = [
            np.zeros((NCORES * z.shape[0], *z.shape[1:]), z.dtype)
            for z in self.zero_outs
        ]
        outs = self.sharded(*args, *zeros)
        return {n: np.asarray(outs[i]) for i, n in enumerate(self.out_names)}


def _get_runner():
    global _RUNNER
    if _RUNNER is None:
        _RUNNER = _Runner()
    return _RUNNER


def pack_inputs(feature, weight):
    """Host-side packing: returns dict of concatenated per-core inputs."""
    feature = np.ascontiguousarray(np.asarray(feature, dtype=np.float32))
    weight = np.ascontiguousarray(np.asarray(weight, dtype=np.float32))
    # featT pack: fpk[core][p*64+e, bb*F+i] = feature[core*BLOC + 2*bb + p, i, e]
    ft = feature.reshape(NCORES, NPAIR, 2, F, E)  # [core, bb, p, i, e]
    fpk = (
        np.ascontiguousarray(ft.transpose(0, 2, 4, 1, 3))
        .reshape(NCORES * 128, NPAIR * F)
        .astype(np.float16)
    )
    wt = np.ascontiguousarray(weight.transpose(1, 0, 2)).reshape(E, F * E)
    wpk_one = np.concatenate([wt, wt], axis=0).astype(np.float16)  # (128, F*E)
    wpk = np.tile(wpk_one, (NCORES, 1))
    return {"fpk": fpk, "wpk": wpk}


def kernel(feature, weight):
    r = _get_runner()
    ins = pack_inputs(feature, weight)
    outs = r.run(ins)
    return unpack_output(outs["out"])


def unpack_output(out_dev):
    """out_dev: (8*100, BLOC, 2, F) device layout -> (B, F, F)."""
    o = out_dev.reshape(NCORES, 100, BLOC, 2, F)
    # out[core, b, 2p+ci, j] = o[core, p, b, ci, j]
    return np.ascontiguousarray(o.transpose(0, 2, 1, 3, 4)).reshape(B, F, F)


if __name__ == "__main__":
    rng = np.random.default_rng(0)
    feature = rng.standard_normal((B, F, E), dtype=np.float32)
    weight = (0.01 * rng.standard_normal((F, E, E))).astype(np.float32)
    got = kernel(feature, weight)
    bil = np.einsum(
        "bie,ied->bid", feature.astype(np.float64), weight.astype(np.float64)
    )
    ref = np.einsum("bid,bjd->bij", bil, feature.astype(np.float64))
    err = np.abs(got - ref)
    denom = np.abs(ref).max()
    print("max abs err:", err.max(), "rel(scale):", err.max() / denom)
    l2 = np.linalg.norm((got - ref).ravel()) / np.linalg.norm(ref.ravel())
    print("L2 rel:", l2)
